# revision 10
# baseline (speedup 1.0000x reference)
"""XNOR-Net++ 3x3 conv (sign(x) (*) sign(w) * alpha*beta*gamma) on 8 TRN2 NeuronCores.

Sharding: data-parallel over batch (32 -> 4 per core), weights/scales replicated.

Per core:
- binarize x and w on-device to fp8e4 (+-1 is exact; PSUM accumulates fp32 exactly)
- ONE width+height padded sign image per slot [128, 2, 58, 58] fp8 (two persistent
  ping-pong slots, borders zeroed once); the 3 kx taps are column offsets in the
  moving AP, so no shifted copies and no per-image memsets
- 3x3 conv = 9 accumulating DoubleRow matmuls per [128, 448] output tile
  (K=256 via input-channel-block pairing, 2 fp8 weights/PE cell)
- weights transposed on-device via PE transpose; pair dim step 128 B (%16==0)
- epilogue: single DVE mul with precomputed abg[p, ob, pix] = alpha*beta*gamma
  (alpha folded into the beta*gamma broadcast via K=1 matmuls)
- output batched per (image, ob) into SBUF, then one 1.6 MB contiguous DMA
"""

from contextlib import ExitStack

import numpy as np

import concourse.bacc as bacc
import concourse.bass as bass
import concourse.mybir as mybir
import concourse.tile as tile
from concourse import masks
from concourse.bass_utils import run_bass_kernel_spmd

N_CORES = 8
B, C, H, KS = 32, 256, 56, 3
P = 128
CB = C // P  # input-channel blocks (2)
OB = C // P  # output-channel blocks (2)
HP = H + 2   # padded image rows (58)
WP = H + 2   # padded image cols (58)
R = 8        # output rows per matmul tile
T = H // R   # row tiles per image (7)
NT = R * H   # moving free dim per matmul (448)
HW = H * H   # pixels per image (3136)

F32 = mybir.dt.float32
BF16 = mybir.dt.bfloat16
FP8 = mybir.dt.float8e4
DR = mybir.MatmulPerfMode.DoubleRow


def build_conv(tc, out_ap, x_ap, w_ap, a_ap, b_ap, g_ap, BL):
    nc = tc.nc
    with ExitStack() as ctx:
        const_pool = ctx.enter_context(tc.tile_pool(name="const", bufs=1))
        wpool = ctx.enter_context(tc.tile_pool(name="w", bufs=1))
        xpool = ctx.enter_context(tc.tile_pool(name="x", bufs=2))
        psumpool = ctx.enter_context(tc.tile_pool(name="psum", bufs=4, space="PSUM"))
        opool = ctx.enter_context(tc.tile_pool(name="o", bufs=4))

        ident = const_pool.tile([P, P], BF16, name="ident")
        masks.make_identity(nc, ident)

        # ---- tiny scale DMAs first (near-zero delay to the w DMA behind them)
        a_t = const_pool.tile([P, OB], F32, name="a_t")
        nc.sync.dma_start(a_t, a_ap.rearrange("(ob p) u v -> p (ob u v)", p=P))
        b_t = const_pool.tile([1, H], F32, name="b_t")
        nc.sync.dma_start(b_t, b_ap[0:1, :, 0])
        g_t = const_pool.tile([1, H], F32, name="g_t")
        nc.sync.dma_start(g_t, g_ap[0:1, 0, :])
        ones_t = const_pool.tile([1, P], F32, name="ones_t")
        nc.gpsimd.memset(ones_t, 1.0)

        # ---- persistent padded sign-image slots; borders zeroed once ----
        imgs = [
            wpool.tile([P, CB, HP, WP], FP8, name=f"img{s}") for s in range(2)
        ]
        nc.gpsimd.memset(imgs[0], 0.0)
        nc.gpsimd.memset(imgs[1], 0.0)

        HROWS = H // 2  # 28
        x_v = x_ap.rearrange("b (cb p) h w -> b p cb (h w)", p=P)

        def emit_load_half(b, h, x_t, dma_engine):
            im = imgs[b % 2]
            rs, re = h * HROWS, (h + 1) * HROWS
            dma_engine.dma_start(
                x_t[:, :, rs * H : re * H], x_v[b][:, :, rs * H : re * H]
            )
            nc.scalar.sign(
                im[:, :, 1 + rs : 1 + re, 1 : H + 1],
                x_t.rearrange("p cb (h w) -> p cb h w", h=H)[:, :, rs:re, :],
            )
            return im

        def emit_load(b, dma_engine):
            x_t = xpool.tile([P, CB, HW], F32, name="x_t")
            for h in range(2):
                im = emit_load_half(b, h, x_t, dma_engine)
            return im

        # ---- weights: DMA + sign split per ob, interleaved with the image-0
        # halves so ACT alternates w-sign / x-sign and the first conv matmul
        # can start as soon as w0 transposes + the first half-sign are done ----
        w_f32 = wpool.tile([P, OB, C * KS * KS], F32, name="w_f32")
        w_sgn = wpool.tile([P, OB, C * KS * KS], BF16, name="w_sgn")
        w_dram = w_ap.rearrange("(ob p) i ky kx -> p ob (i ky kx)", p=P)
        x0_t = xpool.tile([P, CB, HW], F32, name="x_t")
        for ob in range(OB):
            nc.sync.dma_start(w_f32[:, ob], w_dram[:, ob])
            nc.scalar.sign(w_sgn[:, ob], w_f32[:, ob])
            im_cur = emit_load_half(0, ob, x0_t, nc.sync)
        w_view = w_sgn.rearrange("p ob (i kk) -> p ob kk i", kk=KS * KS)

        # broadcast beta/gamma rows to all 128 partitions via tiny K=1 matmuls,
        # then build abg[p, ob, pix] = alpha*beta*gamma on the (idle) DVE
        b_bcast = const_pool.tile([P, H], F32, name="b_bcast")
        g_bcast = const_pool.tile([P, H], F32, name="g_bcast")
        for src, dst in ((b_t, b_bcast), (g_t, g_bcast)):
            bgp = psumpool.tile([P, H], F32, name="bgp", tag="bgp", bufs=1)
            nc.tensor.matmul(bgp, ones_t, src[0:1, :], start=True, stop=True)
            nc.vector.tensor_copy(dst, bgp)

        abg = const_pool.tile([P, OB, HW], F32, name="abg")
        abg_v = abg.rearrange("p o (i j) -> p o i j", i=H)
        ab = const_pool.tile([P, OB, H], F32, name="ab")
        for ob in range(OB):
            nc.vector.tensor_mul(
                ab[:, ob, :], b_bcast, a_t[:, ob : ob + 1].to_broadcast((P, H))
            )
            nc.vector.tensor_mul(
                abg_v[:, ob],
                ab[:, ob, :].unsqueeze(2).to_broadcast((P, H, H)),
                g_bcast.unsqueeze(1).to_broadcast((P, H, H)),
            )

        # wT2[i_low, tap, ob, cb, o] in fp8; pair dim cb has byte-step 128 (%16==0)
        # PSUM->SBUF copies on DVE so ACT stays free for the image signs
        wT2 = wpool.tile([P, KS * KS, OB, CB, P], FP8, name="wT2")
        for ob in range(OB):
            for ib in range(CB):
                for kk in range(KS * KS):
                    pt = psumpool.tile([P, P], BF16, name="pt", tag="pt", bufs=3)
                    nc.tensor.transpose(
                        pt, w_view[:, ob, kk, ib * P : (ib + 1) * P], ident
                    )
                    nc.vector.tensor_copy(wT2[:, kk, ob, ib, :], pt)

        # ---- main loop over local batches ----
        out_v = out_ap.rearrange("b (ob p) h w -> b ob p (h w)", p=P)
        for b in range(BL):
            im = im_cur
            for ob in range(OB):
                o_t = opool.tile([P, HW], F32, name="o_t")
                for t in range(T):
                    ps = psumpool.tile([P, NT], F32, name="cps", tag="cps", bufs=4)
                    for kk in range(KS * KS):
                        ky, kx = divmod(kk, KS)
                        rhs = im[:, :, t * R + ky : t * R + ky + R, kx : kx + H]
                        nc.tensor.matmul(
                            ps,
                            wT2[:, kk, ob, :, :],
                            rhs,
                            start=(kk == 0),
                            stop=(kk == KS * KS - 1),
                            perf_mode=DR,
                        )
                    sl = slice(t * NT, (t + 1) * NT)
                    nc.vector.tensor_mul(o_t[:, sl], ps, abg[:, ob, sl])
                    if ob == 0 and t == 1 and b + 1 < BL:
                        # prefetch next image mid-stream: DMA issued from the
                        # (idle) gpsimd queue so neither the SP queue (startup
                        # w/x DMAs) nor the out DMAs can block it
                        im_cur = emit_load(b + 1, nc.gpsimd)
                    if t in (3, 5):
                        cs = slice(0, 4 * NT) if t == 3 else slice(4 * NT, 6 * NT)
                        nc.scalar.dma_start(out_v[b, ob][:, cs], o_t[:, cs])
                cs = slice(6 * NT, T * NT)
                nc.scalar.dma_start(out_v[b, ob][:, cs], o_t[:, cs])


def build_nc(BL):
    nc = bacc.Bacc("TRN2", target_bir_lowering=False, debug=False)
    x = nc.dram_tensor("x", [BL, C, H, H], F32, kind="ExternalInput")
    w = nc.dram_tensor("weight", [C, C, KS, KS], F32, kind="ExternalInput")
    a = nc.dram_tensor("alpha", [C, 1, 1], F32, kind="ExternalInput")
    be = nc.dram_tensor("beta", [1, H, 1], F32, kind="ExternalInput")
    g = nc.dram_tensor("gamma", [1, 1, H], F32, kind="ExternalInput")
    o = nc.dram_tensor("out", [BL, C, H, H], F32, kind="ExternalOutput")
    with tile.TileContext(nc) as tc:
        build_conv(tc, o.ap(), x.ap(), w.ap(), a.ap(), be.ap(), g.ap(), BL)
    nc.compile()
    return nc


_nc_cache = {}


def _get_nc(BL):
    if BL not in _nc_cache:
        _nc_cache[BL] = build_nc(BL)
    return _nc_cache[BL]


def kernel(x, weight, alpha, beta, gamma):
    x = np.ascontiguousarray(np.asarray(x, dtype=np.float32))
    weight = np.ascontiguousarray(np.asarray(weight, dtype=np.float32))
    alpha = np.ascontiguousarray(np.asarray(alpha, dtype=np.float32))
    beta = np.ascontiguousarray(np.asarray(beta, dtype=np.float32))
    gamma = np.ascontiguousarray(np.asarray(gamma, dtype=np.float32))

    BL = B // N_CORES
    nc = _get_nc(BL)
    xs = x.reshape(N_CORES, BL, C, H, H)
    in_maps = [
        {"x": xs[c], "weight": weight, "alpha": alpha, "beta": beta, "gamma": gamma}
        for c in range(N_CORES)
    ]
    res = run_bass_kernel_spmd(nc, in_maps, list(range(N_CORES)))
    return np.concatenate([r["out"] for r in res.results], axis=0)


# revision 11
# speedup vs baseline: 1.0127x; 1.0127x over previous
"""XNOR-Net++ 3x3 conv (sign(x) (*) sign(w) * alpha*beta*gamma) on 8 TRN2 NeuronCores.

Sharding: data-parallel over batch (32 -> 4 per core), weights/scales replicated.

Per core:
- binarize x and w on-device to fp8e4 (+-1 is exact; PSUM accumulates fp32 exactly)
- ONE width+height padded sign image per slot [128, 2, 58, 58] fp8 (two persistent
  ping-pong slots, borders zeroed once); the 3 kx taps are column offsets in the
  moving AP, so no shifted copies and no per-image memsets
- 3x3 conv = 9 accumulating DoubleRow matmuls per [128, 448] output tile
  (K=256 via input-channel-block pairing, 2 fp8 weights/PE cell)
- weights transposed on-device via PE transpose; pair dim step 128 B (%16==0)
- epilogue: single DVE mul with precomputed abg[p, ob, pix] = alpha*beta*gamma
  (alpha folded into the beta*gamma broadcast via K=1 matmuls)
- output batched per (image, ob) into SBUF, then one 1.6 MB contiguous DMA
"""

from contextlib import ExitStack

import numpy as np

import concourse.bacc as bacc
import concourse.bass as bass
import concourse.mybir as mybir
import concourse.tile as tile
from concourse import masks
from concourse.bass_utils import run_bass_kernel_spmd

N_CORES = 8
B, C, H, KS = 32, 256, 56, 3
P = 128
CB = C // P  # input-channel blocks (2)
OB = C // P  # output-channel blocks (2)
HP = H + 2   # padded image rows (58)
WP = H + 2   # padded image cols (58)
R = 8        # output rows per matmul tile
T = H // R   # row tiles per image (7)
NT = R * H   # moving free dim per matmul (448)
HW = H * H   # pixels per image (3136)

F32 = mybir.dt.float32
BF16 = mybir.dt.bfloat16
FP8 = mybir.dt.float8e4
DR = mybir.MatmulPerfMode.DoubleRow


def build_conv(tc, out_ap, x_ap, w_ap, a_ap, b_ap, g_ap, BL):
    nc = tc.nc
    with ExitStack() as ctx:
        const_pool = ctx.enter_context(tc.tile_pool(name="const", bufs=1))
        wpool = ctx.enter_context(tc.tile_pool(name="w", bufs=1))
        xpool = ctx.enter_context(tc.tile_pool(name="x", bufs=2))
        psumpool = ctx.enter_context(tc.tile_pool(name="psum", bufs=4, space="PSUM"))
        opool = ctx.enter_context(tc.tile_pool(name="o", bufs=4))

        ident = const_pool.tile([P, P], BF16, name="ident")
        masks.make_identity(nc, ident)

        # ---- tiny scale DMAs first (near-zero delay to the w DMA behind them)
        a_t = const_pool.tile([P, OB], F32, name="a_t")
        nc.sync.dma_start(a_t, a_ap.rearrange("(ob p) u v -> p (ob u v)", p=P))
        b_t = const_pool.tile([1, H], F32, name="b_t")
        nc.sync.dma_start(b_t, b_ap[0:1, :, 0])
        g_t = const_pool.tile([1, H], F32, name="g_t")
        nc.sync.dma_start(g_t, g_ap[0:1, 0, :])
        ones_t = const_pool.tile([1, P], F32, name="ones_t")
        nc.gpsimd.memset(ones_t, 1.0)

        # ---- persistent padded sign-image slots; borders zeroed once ----
        imgs = [
            wpool.tile([P, CB, HP, WP], FP8, name=f"img{s}") for s in range(2)
        ]
        nc.gpsimd.memset(imgs[0], 0.0)
        nc.gpsimd.memset(imgs[1], 0.0)

        HROWS = H // 2  # 28
        x_v = x_ap.rearrange("b (cb p) h w -> b p cb (h w)", p=P)

        def emit_dma_half(b, h, x_t, dma_engine):
            rs, re = h * HROWS, (h + 1) * HROWS
            dma_engine.dma_start(
                x_t[:, :, rs * H : re * H], x_v[b][:, :, rs * H : re * H]
            )

        def emit_sign_half(b, h, x_t):
            im = imgs[b % 2]
            rs, re = h * HROWS, (h + 1) * HROWS
            nc.scalar.sign(
                im[:, :, 1 + rs : 1 + re, 1 : H + 1],
                x_t.rearrange("p cb (h w) -> p cb h w", h=H)[:, :, rs:re, :],
            )
            return im

        def emit_load(b, dma_engine):
            x_t = xpool.tile([P, CB, HW], F32, name="x_t")
            for h in range(2):
                emit_dma_half(b, h, x_t, dma_engine)
                im = emit_sign_half(b, h, x_t)
            return im

        # ---- weights: DMA + sign split per ob. The x0 DMAs are issued from
        # the ACT queue right after w0's sign, so the weight DMAs get the full
        # HBM bandwidth first and x0 still lands in time for its signs ----
        w_f32 = wpool.tile([P, OB, C * KS * KS], F32, name="w_f32")
        w_sgn = wpool.tile([P, OB, C * KS * KS], BF16, name="w_sgn")
        w_dram = w_ap.rearrange("(ob p) i ky kx -> p ob (i ky kx)", p=P)
        x0_t = xpool.tile([P, CB, HW], F32, name="x_t")
        for ob in range(OB):
            nc.sync.dma_start(w_f32[:, ob], w_dram[:, ob])
            nc.scalar.sign(w_sgn[:, ob], w_f32[:, ob])
            if ob == 0:
                emit_dma_half(0, 0, x0_t, nc.scalar)
                emit_dma_half(0, 1, x0_t, nc.scalar)
        im_cur = emit_sign_half(0, 0, x0_t)
        im_cur = emit_sign_half(0, 1, x0_t)
        w_view = w_sgn.rearrange("p ob (i kk) -> p ob kk i", kk=KS * KS)

        # broadcast beta/gamma rows to all 128 partitions via tiny K=1 matmuls,
        # then build abg[p, ob, pix] = alpha*beta*gamma on the (idle) DVE
        b_bcast = const_pool.tile([P, H], F32, name="b_bcast")
        g_bcast = const_pool.tile([P, H], F32, name="g_bcast")
        for src, dst in ((b_t, b_bcast), (g_t, g_bcast)):
            bgp = psumpool.tile([P, H], F32, name="bgp", tag="bgp", bufs=1)
            nc.tensor.matmul(bgp, ones_t, src[0:1, :], start=True, stop=True)
            nc.vector.tensor_copy(dst, bgp)

        abg = const_pool.tile([P, OB, HW], F32, name="abg")
        abg_v = abg.rearrange("p o (i j) -> p o i j", i=H)
        ab = const_pool.tile([P, OB, H], F32, name="ab")
        for ob in range(OB):
            nc.vector.tensor_mul(
                ab[:, ob, :], b_bcast, a_t[:, ob : ob + 1].to_broadcast((P, H))
            )
            nc.vector.tensor_mul(
                abg_v[:, ob],
                ab[:, ob, :].unsqueeze(2).to_broadcast((P, H, H)),
                g_bcast.unsqueeze(1).to_broadcast((P, H, H)),
            )

        # wT2[i_low, tap, ob, cb, o] in fp8; pair dim cb has byte-step 128 (%16==0)
        # PSUM->SBUF copies on DVE so ACT stays free for the image signs
        wT2 = wpool.tile([P, KS * KS, OB, CB, P], FP8, name="wT2")
        for ob in range(OB):
            for ib in range(CB):
                for kk in range(KS * KS):
                    pt = psumpool.tile([P, P], BF16, name="pt", tag="pt", bufs=3)
                    nc.tensor.transpose(
                        pt, w_view[:, ob, kk, ib * P : (ib + 1) * P], ident
                    )
                    nc.vector.tensor_copy(wT2[:, kk, ob, ib, :], pt)

        # ---- main loop over local batches ----
        out_v = out_ap.rearrange("b (ob p) h w -> b ob p (h w)", p=P)
        for b in range(BL):
            im = im_cur
            for ob in range(OB):
                o_t = opool.tile([P, HW], F32, name="o_t")
                for t in range(T):
                    ps = psumpool.tile([P, NT], F32, name="cps", tag="cps", bufs=4)
                    for kk in range(KS * KS):
                        ky, kx = divmod(kk, KS)
                        rhs = im[:, :, t * R + ky : t * R + ky + R, kx : kx + H]
                        nc.tensor.matmul(
                            ps,
                            wT2[:, kk, ob, :, :],
                            rhs,
                            start=(kk == 0),
                            stop=(kk == KS * KS - 1),
                            perf_mode=DR,
                        )
                    sl = slice(t * NT, (t + 1) * NT)
                    nc.vector.tensor_mul(o_t[:, sl], ps, abg[:, ob, sl])
                    if ob == 0 and t == 1 and b + 1 < BL:
                        # prefetch next image mid-stream: DMA issued from the
                        # (idle) gpsimd queue so neither the SP queue (startup
                        # w/x DMAs) nor the out DMAs can block it
                        im_cur = emit_load(b + 1, nc.gpsimd)
                    if t in (3, 5):
                        cs = slice(0, 4 * NT) if t == 3 else slice(4 * NT, 6 * NT)
                        nc.scalar.dma_start(out_v[b, ob][:, cs], o_t[:, cs])
                cs = slice(6 * NT, T * NT)
                nc.scalar.dma_start(out_v[b, ob][:, cs], o_t[:, cs])


def build_nc(BL):
    nc = bacc.Bacc("TRN2", target_bir_lowering=False, debug=False)
    x = nc.dram_tensor("x", [BL, C, H, H], F32, kind="ExternalInput")
    w = nc.dram_tensor("weight", [C, C, KS, KS], F32, kind="ExternalInput")
    a = nc.dram_tensor("alpha", [C, 1, 1], F32, kind="ExternalInput")
    be = nc.dram_tensor("beta", [1, H, 1], F32, kind="ExternalInput")
    g = nc.dram_tensor("gamma", [1, 1, H], F32, kind="ExternalInput")
    o = nc.dram_tensor("out", [BL, C, H, H], F32, kind="ExternalOutput")
    with tile.TileContext(nc) as tc:
        build_conv(tc, o.ap(), x.ap(), w.ap(), a.ap(), be.ap(), g.ap(), BL)
    nc.compile()
    return nc


_nc_cache = {}


def _get_nc(BL):
    if BL not in _nc_cache:
        _nc_cache[BL] = build_nc(BL)
    return _nc_cache[BL]


def kernel(x, weight, alpha, beta, gamma):
    x = np.ascontiguousarray(np.asarray(x, dtype=np.float32))
    weight = np.ascontiguousarray(np.asarray(weight, dtype=np.float32))
    alpha = np.ascontiguousarray(np.asarray(alpha, dtype=np.float32))
    beta = np.ascontiguousarray(np.asarray(beta, dtype=np.float32))
    gamma = np.ascontiguousarray(np.asarray(gamma, dtype=np.float32))

    BL = B // N_CORES
    nc = _get_nc(BL)
    xs = x.reshape(N_CORES, BL, C, H, H)
    in_maps = [
        {"x": xs[c], "weight": weight, "alpha": alpha, "beta": beta, "gamma": gamma}
        for c in range(N_CORES)
    ]
    res = run_bass_kernel_spmd(nc, in_maps, list(range(N_CORES)))
    return np.concatenate([r["out"] for r in res.results], axis=0)
